# revision 12
# baseline (speedup 1.0000x reference)
"""Trainium2 Bass kernel for suffix-softmax attention visualization.

Computes, for hidden_states [S, B, H], W [H, 1], b [1]:
    s[t, b]   = sum_h hidden_states[t, b, h] * W[h, 0] + b[0]
    out[t, b] = exp(s[t, b]) / sum_{t' >= t} exp(s[t', b])     (suffix softmax)
returned as [S, B, 1] f32.  The softmax ratio is shift-invariant so b cancels.

Sharding: data-parallel over batch — 8 NeuronCores, 8 batch columns each.

v5 design (memory regime; per-core floor = 64 MiB f32 HBM reads at ~375 GB/s):
  - Seq blocks are processed in REVERSE order (suffix scan) with a running
    esum[p, b]; per block the suffix denominator = tri @ e + ones @ esum
    lands in PSUM via two tiny PE matmuls — no scan epilogue, streaming
    output chunks.
  - Middle blocks (j=1..30) stream via SWDGE (gpsimd) DMAs with an inline
    f32->fp16 cast; fp16 unlocks the DVE 2x_1p tensor_tensor multiply
    (one instruction per block), and the h-reduction splits between ScalarE
    activation(Copy, accum_out) (5/4 cols alternating) and one grouped
    VectorE tensor_reduce (3/4 cols).
  - The first and last processed blocks ride the otherwise-idle HWDGE
    rings as f32 and are reduced by DVE scalar_tensor_tensor directly:
    j=0 fills the ramp while the SWDGE queue boots; j=31's scores/exp are
    computed mid-stream (its DMA landed early), so after the last SWDGE
    byte only its tiny scan chain (~3 us) remains.
  - Output is written [128 p, 32 j, 8 b] (j = reversed block index,
    contiguous per partition); the host unscrambles to [4096, 8].
"""

import numpy as np

import concourse.bacc as bacc
import concourse.mybir as mybir
import concourse.tile as tile
from concourse import bass_utils

P = 128
S = 4096
B = 64
H = 512
N_CORES = 8
BC = B // N_CORES
NBLK = S // P
C = NBLK * BC

F32 = mybir.dt.float32
F16 = mybir.dt.float16

Copy = mybir.ActivationFunctionType.Copy
Exp = mybir.ActivationFunctionType.Exp
X = mybir.AxisListType.X
MULT = mybir.AluOpType.mult


def build_program():
    nc = bacc.Bacc("TRN2", target_bir_lowering=False, debug=False)
    hs = nc.dram_tensor("hs", [S, BC, H], F32, kind="ExternalInput")
    wrep = nc.dram_tensor("wrep", [P, BC * H], F16, kind="ExternalInput")
    wbc = nc.dram_tensor("wbc", [P, H], F32, kind="ExternalInput")
    tri = nc.dram_tensor("tri", [P, P], F32, kind="ExternalInput")
    onesq = nc.dram_tensor("onesq", [P, P], F32, kind="ExternalInput")
    out = nc.dram_tensor("out", [P, C], F32, kind="ExternalOutput")

    with tile.TileContext(nc) as tc:
        with (
            tc.tile_pool(name="hsp", bufs=10) as hsp,
            tc.tile_pool(name="hsf32", bufs=2) as hsf32,
            tc.tile_pool(name="prodp", bufs=3) as prodp,
            tc.tile_pool(name="consts", bufs=1) as consts,
            tc.tile_pool(name="scolp", bufs=4) as scolp,
            tc.tile_pool(name="ep", bufs=4) as ep,
            tc.tile_pool(name="recp", bufs=4) as recp,
            tc.tile_pool(name="work", bufs=1) as work,
            tc.tile_pool(name="psum", bufs=6, space="PSUM") as psum,
        ):
            hs_ap = hs.ap()

            def blk_rows(j):
                k = NBLK - 1 - j
                return hs_ap[k * P : (k + 1) * P, :, :]

            # HWDGE rings crawl (~50-100 GB/s) once the SWDGE flood starts,
            # so the 1 MiB fp16 wrep rides the SWDGE queue itself, in front
            # of the stream (lands ~12 us).  Only the tiny wbc and the
            # late-needed hst31 prefetch use the sync ring; tri/onesq ride
            # the scalar ring ahead of everything they gate.
            wrep_t = consts.tile([P, BC * H], F16)
            nc.gpsimd.dma_start(out=wrep_t, in_=wrep.ap())
            wb_t = consts.tile([P, H], F32)
            nc.sync.dma_start(out=wb_t, in_=wbc.ap())
            hst31 = hsf32.tile([P, BC, H], F32, name="hs_f32")
            nc.sync.dma_start(out=hst31, in_=blk_rows(31))
            tri_t = consts.tile([P, P], F32)
            nc.scalar.dma_start(out=tri_t, in_=tri.ap())
            onesq_t = consts.tile([P, P], F32)
            nc.scalar.dma_start(out=onesq_t, in_=onesq.ap())
            wrep_v = wrep_t.rearrange("p (b h) -> p b h", h=H)

            # SWDGE fp16 stream; j=0 split into two 4-col chunks for ramp.
            h2 = BC // 2
            dmas = {31: hst31}
            hst0 = hsp.tile([P, BC, H], F16, name="hst")
            nc.gpsimd.dma_start(out=hst0[:, :h2, :], in_=blk_rows(0)[:, :h2, :])
            nc.gpsimd.dma_start(out=hst0[:, h2:, :], in_=blk_rows(0)[:, h2:, :])
            dmas[0] = hst0
            for j in range(1, NBLK - 1):
                hst = hsp.tile([P, BC, H], F16, name="hst")
                nc.gpsimd.dma_start(out=hst, in_=blk_rows(j))
                dmas[j] = hst

            dummy = work.tile([P, 1], F32)
            esum = [
                work.tile([P, BC], F32, name="esum0"),
                work.tile([P, BC], F32, name="esum1"),
            ]
            nc.vector.memset(esum[0], 0.0)
            sel = work.tile([P, C], F32)
            s31 = work.tile([P, BC], F32)
            e31 = work.tile([P, BC], F32)
            out_ap = out.ap()

            def stt_cols(hst, s_col, cols):
                """f32 per-column mult+accum on DVE."""
                for c in cols:
                    nc.vector.scalar_tensor_tensor(
                        out=dummy.broadcast_to((P, H)),
                        in0=hst[:, c, :], scalar=1.0, in1=wb_t,
                        op0=MULT, op1=MULT,
                        accum_out=s_col[:, c : c + 1],
                    )

            def scan_cols(j, e_t, lo, hi, last=False):
                """Scan chain for cols [lo,hi) of processed block j."""
                n = hi - lo
                ps = psum.tile([P, n], F32, name="ps")
                nc.tensor.matmul(ps, tri_t, e_t[:, lo:hi], start=True, stop=False)
                nc.tensor.matmul(
                    ps, onesq_t, esum[j % 2][:, lo:hi], start=False, stop=True
                )
                if not last:
                    nc.vector.tensor_add(
                        esum[(j + 1) % 2][:, lo:hi], esum[j % 2][:, lo:hi],
                        e_t[:, lo:hi],
                    )
                rec = recp.tile([P, n], F32, name="rec")
                nc.vector.reciprocal(rec, ps)
                c0 = j * BC + lo
                nc.vector.tensor_mul(sel[:, c0 : c0 + n], e_t[:, lo:hi], rec)

            prods = {}

            def emit_mult(j, lo=0, hi=BC):
                prod = prodp.tile([P, BC, H], F16, name="prod")
                nc.vector.tensor_tensor(
                    prod[:, lo:hi, :], dmas[j][:, lo:hi, :], wrep_v[:, lo:hi, :],
                    op=MULT,
                )
                prods[j] = prod

            def process_f16_block(j, act_cols):
                prod = prods.pop(j)
                s_col = scolp.tile([P, BC], F32, name="s_col")
                for b in range(act_cols):
                    nc.scalar.activation(
                        dummy.broadcast_to((P, H)),
                        prod[:, b, :],
                        Copy,
                        accum_out=s_col[:, b : b + 1],
                    )
                if act_cols < BC:
                    nc.vector.reduce_sum(
                        out=s_col[:, act_cols:],
                        in_=prod[:, act_cols:, :],
                        axis=X,
                    )
                e_t = ep.tile([P, BC], F32, name="e_t")
                nc.scalar.activation(e_t, s_col, Exp)
                scan_cols(j, e_t, 0, BC)

            # j=0 ramp block: fp16, processed in two 4-col pieces so ACT/DVE
            # start on the first landed chunk.
            emit_mult(0, 0, h2)
            prod0 = prods[0]
            s_col0 = scolp.tile([P, BC], F32, name="s_col")
            e_t0 = ep.tile([P, BC], F32, name="e_t")
            for pc in range(2):
                lo, hi = pc * h2, (pc + 1) * h2
                if pc:
                    nc.vector.tensor_tensor(
                        prod0[:, lo:hi, :], dmas[0][:, lo:hi, :],
                        wrep_v[:, lo:hi, :], op=MULT,
                    )
                for b in range(lo, lo + 2):
                    nc.scalar.activation(
                        dummy.broadcast_to((P, H)), prod0[:, b, :], Copy,
                        accum_out=s_col0[:, b : b + 1],
                    )
                nc.vector.reduce_sum(
                    out=s_col0[:, lo + 2 : hi], in_=prod0[:, lo + 2 : hi, :], axis=X
                )
                nc.scalar.activation(e_t0[:, lo:hi], s_col0[:, lo:hi], Exp)
                scan_cols(0, e_t0, lo, hi)
            prods.pop(0)

            # middle fp16 blocks, multiplies emitted two blocks ahead so the
            # next block's ACT accums never queue behind this block's scan.
            emit_mult(1)
            emit_mult(2)
            for j in range(1, NBLK - 1):
                if j + 2 <= NBLK - 2:
                    emit_mult(j + 2)
                process_f16_block(j, act_cols=5 if j % 2 else 4)
                if j in (20, 22, 24, 26):
                    lo = (j - 20)
                    stt_cols(hst31, s31, range(lo, lo + 2))
                if j == 26:
                    nc.scalar.activation(e31, s31, Exp)

            # j=31 tail: only the scan chain remains.
            scan_cols(31, e31, 0, BC, last=True)

            for glo, ghi in ((0, 16), (16, 24), (24, 30), (30, 32)):
                lo, hi = glo * BC, ghi * BC
                nc.sync.dma_start(out=out_ap[:, lo:hi], in_=sel[:, lo:hi])

    nc.compile()
    return nc


_PROGRAM = None


def _get_program():
    global _PROGRAM
    if _PROGRAM is None:
        _PROGRAM = build_program()
    return _PROGRAM


def make_in_maps(hidden_states, W):
    hidden_states = np.asarray(hidden_states, dtype=np.float32)
    w = np.asarray(W, dtype=np.float32)[:, 0]
    w16 = w.astype(np.float16)
    wrep = np.ascontiguousarray(np.tile(w16[None, :], (P, BC)))
    wbc = np.ascontiguousarray(np.broadcast_to(w[None, :], (P, H))).astype(np.float32)
    tri = np.tril(np.ones((P, P), dtype=np.float32))
    onesq = np.ones((P, P), dtype=np.float32)
    in_maps = []
    for c in range(N_CORES):
        hs_c = np.ascontiguousarray(hidden_states[:, c * BC : (c + 1) * BC, :])
        in_maps.append(
            {"hs": hs_c, "wrep": wrep, "wbc": wbc, "tri": tri, "onesq": onesq}
        )
    return in_maps


def assemble_output(results):
    cols = []
    for c in range(N_CORES):
        oc = results[c]["out"]
        full = oc.reshape(P, NBLK, BC)[:, ::-1, :].transpose(1, 0, 2).reshape(S, BC)
        cols.append(full)
    return np.concatenate(cols, axis=1)[..., None].astype(np.float32)


def kernel(hidden_states, W, b):
    nc = _get_program()
    in_maps = make_in_maps(hidden_states, W)
    res = bass_utils.run_bass_kernel_spmd(nc, in_maps, core_ids=list(range(N_CORES)))
    return assemble_output(res.results)


# revision 13
# speedup vs baseline: 1.0819x; 1.0819x over previous
"""Trainium2 Bass kernel for suffix-softmax attention visualization.

Computes, for hidden_states [S, B, H], W [H, 1], b [1]:
    s[t, b]   = sum_h hidden_states[t, b, h] * W[h, 0] + b[0]
    out[t, b] = exp(s[t, b]) / sum_{t' >= t} exp(s[t', b])     (suffix softmax)
returned as [S, B, 1] f32.  The softmax ratio is shift-invariant so b cancels.

Sharding: data-parallel over batch — 8 NeuronCores, 8 batch columns each.

v5 design (memory regime; per-core floor = 64 MiB f32 HBM reads at ~375 GB/s):
  - Seq blocks are processed in REVERSE order (suffix scan) with a running
    esum[p, b]; per block the suffix denominator = tri @ e + ones @ esum
    lands in PSUM via two tiny PE matmuls — no scan epilogue, streaming
    output chunks.
  - Middle blocks (j=1..30) stream via SWDGE (gpsimd) DMAs with an inline
    f32->fp16 cast; fp16 unlocks the DVE 2x_1p tensor_tensor multiply
    (one instruction per block), and the h-reduction splits between ScalarE
    activation(Copy, accum_out) (5/4 cols alternating) and one grouped
    VectorE tensor_reduce (3/4 cols).
  - The first and last processed blocks ride the otherwise-idle HWDGE
    rings as f32 and are reduced by DVE scalar_tensor_tensor directly:
    j=0 fills the ramp while the SWDGE queue boots; j=31's scores/exp are
    computed mid-stream (its DMA landed early), so after the last SWDGE
    byte only its tiny scan chain (~3 us) remains.
  - Output is written [128 p, 32 j, 8 b] (j = reversed block index,
    contiguous per partition); the host unscrambles to [4096, 8].
"""

import numpy as np

import concourse.bacc as bacc
import concourse.mybir as mybir
import concourse.tile as tile
from concourse import bass_utils

P = 128
S = 4096
B = 64
H = 512
N_CORES = 8
BC = B // N_CORES
NBLK = S // P
C = NBLK * BC

F32 = mybir.dt.float32
F16 = mybir.dt.float16

Copy = mybir.ActivationFunctionType.Copy
Exp = mybir.ActivationFunctionType.Exp
X = mybir.AxisListType.X
MULT = mybir.AluOpType.mult


def build_program():
    nc = bacc.Bacc("TRN2", target_bir_lowering=False, debug=False)
    hs = nc.dram_tensor("hs", [S, BC, H], F32, kind="ExternalInput")
    wrep = nc.dram_tensor("wrep", [P, BC * H], F16, kind="ExternalInput")
    wbc = nc.dram_tensor("wbc", [P, H], F32, kind="ExternalInput")
    tri = nc.dram_tensor("tri", [P, P], F32, kind="ExternalInput")
    onesq = nc.dram_tensor("onesq", [P, P], F32, kind="ExternalInput")
    out = nc.dram_tensor("out", [P, C], F32, kind="ExternalOutput")

    with tile.TileContext(nc) as tc:
        with (
            tc.tile_pool(name="hsp", bufs=12) as hsp,
            tc.tile_pool(name="hsf32", bufs=2) as hsf32,
            tc.tile_pool(name="prodp", bufs=3) as prodp,
            tc.tile_pool(name="consts", bufs=1) as consts,
            tc.tile_pool(name="scolp", bufs=4) as scolp,
            tc.tile_pool(name="ep", bufs=4) as ep,
            tc.tile_pool(name="recp", bufs=4) as recp,
            tc.tile_pool(name="work", bufs=1) as work,
            tc.tile_pool(name="psum", bufs=6, space="PSUM") as psum,
        ):
            hs_ap = hs.ap()

            def blk_rows(j):
                k = NBLK - 1 - j
                return hs_ap[k * P : (k + 1) * P, :, :]

            # HWDGE rings crawl (~50-100 GB/s) once the SWDGE flood starts,
            # so the 1 MiB fp16 wrep rides the SWDGE queue itself, in front
            # of the stream (lands ~12 us).  Only the tiny wbc and the
            # late-needed hst31 prefetch use the sync ring; tri/onesq ride
            # the scalar ring ahead of everything they gate.
            wrep_t = consts.tile([P, BC * H], F16)
            nc.gpsimd.dma_start(out=wrep_t, in_=wrep.ap())
            wb_t = consts.tile([P, H], F32)
            nc.sync.dma_start(out=wb_t, in_=wbc.ap())
            hst31 = hsf32.tile([P, BC, H], F32, name="hs_f32")
            nc.sync.dma_start(out=hst31, in_=blk_rows(31))
            tri_t = consts.tile([P, P], F32)
            nc.scalar.dma_start(out=tri_t, in_=tri.ap())
            onesq_t = consts.tile([P, P], F32)
            nc.scalar.dma_start(out=onesq_t, in_=onesq.ap())
            wrep_v = wrep_t.rearrange("p (b h) -> p b h", h=H)

            # SWDGE fp16 stream; j=0 split into two 4-col chunks for ramp.
            h2 = BC // 2
            dmas = {31: hst31}
            hst0 = hsp.tile([P, BC, H], F16, name="hst")
            nc.gpsimd.dma_start(out=hst0[:, :h2, :], in_=blk_rows(0)[:, :h2, :])
            nc.gpsimd.dma_start(out=hst0[:, h2:, :], in_=blk_rows(0)[:, h2:, :])
            dmas[0] = hst0
            for j in range(1, NBLK - 1):
                hst = hsp.tile([P, BC, H], F16, name="hst")
                nc.gpsimd.dma_start(out=hst, in_=blk_rows(j))
                dmas[j] = hst

            dummy = work.tile([P, 1], F32)
            esum = [
                work.tile([P, BC], F32, name="esum0"),
                work.tile([P, BC], F32, name="esum1"),
            ]
            nc.vector.memset(esum[0], 0.0)
            sel = work.tile([P, C], F32)
            s31 = work.tile([P, BC], F32)
            e31 = work.tile([P, BC], F32)
            out_ap = out.ap()

            def stt_cols(hst, s_col, cols):
                """f32 per-column mult+accum on DVE."""
                for c in cols:
                    nc.vector.scalar_tensor_tensor(
                        out=dummy.broadcast_to((P, H)),
                        in0=hst[:, c, :], scalar=1.0, in1=wb_t,
                        op0=MULT, op1=MULT,
                        accum_out=s_col[:, c : c + 1],
                    )

            def scan_cols(j, e_t, lo, hi, last=False):
                """Scan chain for cols [lo,hi) of processed block j."""
                n = hi - lo
                ps = psum.tile([P, n], F32, name="ps")
                nc.tensor.matmul(ps, tri_t, e_t[:, lo:hi], start=True, stop=False)
                nc.tensor.matmul(
                    ps, onesq_t, esum[j % 2][:, lo:hi], start=False, stop=True
                )
                if not last:
                    nc.vector.tensor_add(
                        esum[(j + 1) % 2][:, lo:hi], esum[j % 2][:, lo:hi],
                        e_t[:, lo:hi],
                    )
                rec = recp.tile([P, n], F32, name="rec")
                nc.vector.reciprocal_approx_fast(out=rec, in_=ps)
                c0 = j * BC + lo
                nc.vector.tensor_mul(sel[:, c0 : c0 + n], e_t[:, lo:hi], rec)

            prods = {}

            def emit_mult(j, lo=0, hi=BC):
                prod = prodp.tile([P, BC, H], F16, name="prod")
                nc.vector.tensor_tensor(
                    prod[:, lo:hi, :], dmas[j][:, lo:hi, :], wrep_v[:, lo:hi, :],
                    op=MULT,
                )
                prods[j] = prod

            def process_f16_block(j, act_cols):
                prod = prods.pop(j)
                s_col = scolp.tile([P, BC], F32, name="s_col")
                for b in range(act_cols):
                    nc.scalar.activation(
                        dummy.broadcast_to((P, H)),
                        prod[:, b, :],
                        Copy,
                        accum_out=s_col[:, b : b + 1],
                    )
                if act_cols < BC:
                    nc.vector.reduce_sum(
                        out=s_col[:, act_cols:],
                        in_=prod[:, act_cols:, :],
                        axis=X,
                    )
                e_t = ep.tile([P, BC], F32, name="e_t")
                nc.scalar.activation(e_t, s_col, Exp)
                scan_cols(j, e_t, 0, BC)

            # j=0 ramp block: fp16, processed in two 4-col pieces so ACT/DVE
            # start on the first landed chunk.
            emit_mult(0, 0, h2)
            prod0 = prods[0]
            s_col0 = scolp.tile([P, BC], F32, name="s_col")
            e_t0 = ep.tile([P, BC], F32, name="e_t")
            for pc in range(2):
                lo, hi = pc * h2, (pc + 1) * h2
                if pc:
                    nc.vector.tensor_tensor(
                        prod0[:, lo:hi, :], dmas[0][:, lo:hi, :],
                        wrep_v[:, lo:hi, :], op=MULT,
                    )
                for b in range(lo, lo + 2):
                    nc.scalar.activation(
                        dummy.broadcast_to((P, H)), prod0[:, b, :], Copy,
                        accum_out=s_col0[:, b : b + 1],
                    )
                nc.vector.reduce_sum(
                    out=s_col0[:, lo + 2 : hi], in_=prod0[:, lo + 2 : hi, :], axis=X
                )
                nc.scalar.activation(e_t0[:, lo:hi], s_col0[:, lo:hi], Exp)
                scan_cols(0, e_t0, lo, hi)
            prods.pop(0)

            # middle fp16 blocks, multiplies emitted two blocks ahead so the
            # next block's ACT accums never queue behind this block's scan.
            emit_mult(1)
            emit_mult(2)
            for j in range(1, NBLK - 1):
                if j + 2 <= NBLK - 2:
                    emit_mult(j + 2)
                process_f16_block(j, act_cols=5 if j % 2 else 4)
                if j in (20, 22, 24, 26):
                    lo = (j - 20)
                    stt_cols(hst31, s31, range(lo, lo + 2))
                if j == 26:
                    nc.scalar.activation(e31, s31, Exp)

            # j=31 tail: only the scan chain remains.
            scan_cols(31, e31, 0, BC, last=True)

            for glo, ghi in ((0, 16), (16, 24), (24, 30), (30, 32)):
                lo, hi = glo * BC, ghi * BC
                nc.sync.dma_start(out=out_ap[:, lo:hi], in_=sel[:, lo:hi])

    nc.compile()
    return nc


_PROGRAM = None


def _get_program():
    global _PROGRAM
    if _PROGRAM is None:
        _PROGRAM = build_program()
    return _PROGRAM


def make_in_maps(hidden_states, W):
    hidden_states = np.asarray(hidden_states, dtype=np.float32)
    w = np.asarray(W, dtype=np.float32)[:, 0]
    w16 = w.astype(np.float16)
    wrep = np.ascontiguousarray(np.tile(w16[None, :], (P, BC)))
    wbc = np.ascontiguousarray(np.broadcast_to(w[None, :], (P, H))).astype(np.float32)
    tri = np.tril(np.ones((P, P), dtype=np.float32))
    onesq = np.ones((P, P), dtype=np.float32)
    in_maps = []
    for c in range(N_CORES):
        hs_c = np.ascontiguousarray(hidden_states[:, c * BC : (c + 1) * BC, :])
        in_maps.append(
            {"hs": hs_c, "wrep": wrep, "wbc": wbc, "tri": tri, "onesq": onesq}
        )
    return in_maps


def assemble_output(results):
    cols = []
    for c in range(N_CORES):
        oc = results[c]["out"]
        full = oc.reshape(P, NBLK, BC)[:, ::-1, :].transpose(1, 0, 2).reshape(S, BC)
        cols.append(full)
    return np.concatenate(cols, axis=1)[..., None].astype(np.float32)


def kernel(hidden_states, W, b):
    nc = _get_program()
    in_maps = make_in_maps(hidden_states, W)
    res = bass_utils.run_bass_kernel_spmd(nc, in_maps, core_ids=list(range(N_CORES)))
    return assemble_output(res.results)


# revision 15
# speedup vs baseline: 1.2037x; 1.1125x over previous
"""Trainium2 Bass kernel for suffix-softmax attention visualization.

Computes, for hidden_states [S, B, H], W [H, 1], b [1]:
    s[t, b]   = sum_h hidden_states[t, b, h] * W[h, 0] + b[0]
    out[t, b] = exp(s[t, b]) / sum_{t' >= t} exp(s[t', b])     (suffix softmax)
returned as [S, B, 1] f32.  The softmax ratio is shift-invariant so b cancels.

Sharding: data-parallel over batch — 8 NeuronCores, 8 batch columns each.

v5 design (memory regime; per-core floor = 64 MiB f32 HBM reads at ~375 GB/s):
  - Seq blocks are processed in REVERSE order (suffix scan) with a running
    esum[p, b]; per block the suffix denominator = tri @ e + ones @ esum
    lands in PSUM via two tiny PE matmuls — no scan epilogue, streaming
    output chunks.
  - Middle blocks (j=1..30) stream via SWDGE (gpsimd) DMAs with an inline
    f32->fp16 cast; fp16 unlocks the DVE 2x_1p tensor_tensor multiply
    (one instruction per block), and the h-reduction splits between ScalarE
    activation(Copy, accum_out) (5/4 cols alternating) and one grouped
    VectorE tensor_reduce (3/4 cols).
  - The first and last processed blocks ride the otherwise-idle HWDGE
    rings as f32 and are reduced by DVE scalar_tensor_tensor directly:
    j=0 fills the ramp while the SWDGE queue boots; j=31's scores/exp are
    computed mid-stream (its DMA landed early), so after the last SWDGE
    byte only its tiny scan chain (~3 us) remains.
  - Output is written [128 p, 32 j, 8 b] (j = reversed block index,
    contiguous per partition); the host unscrambles to [4096, 8].
"""

import numpy as np

import concourse.bacc as bacc
import concourse.mybir as mybir
import concourse.tile as tile
from concourse import bass_utils

P = 128
S = 4096
B = 64
H = 512
N_CORES = 8
BC = B // N_CORES
NBLK = S // P
C = NBLK * BC

F32 = mybir.dt.float32
F16 = mybir.dt.float16

Copy = mybir.ActivationFunctionType.Copy
Exp = mybir.ActivationFunctionType.Exp
X = mybir.AxisListType.X
MULT = mybir.AluOpType.mult


def build_program():
    nc = bacc.Bacc("TRN2", target_bir_lowering=False, debug=False)
    hs = nc.dram_tensor("hs", [S, BC, H], F32, kind="ExternalInput")
    wrep = nc.dram_tensor("wrep", [P, BC * H], F16, kind="ExternalInput")
    tri = nc.dram_tensor("tri", [P, P], F32, kind="ExternalInput")
    onesq = nc.dram_tensor("onesq", [P, P], F32, kind="ExternalInput")
    out = nc.dram_tensor("out", [P, C], F32, kind="ExternalOutput")

    with tile.TileContext(nc) as tc:
        with (
            tc.tile_pool(name="hsp", bufs=12) as hsp,
            tc.tile_pool(name="prodp", bufs=3) as prodp,
            tc.tile_pool(name="consts", bufs=1) as consts,
            tc.tile_pool(name="scolp", bufs=4) as scolp,
            tc.tile_pool(name="ep", bufs=4) as ep,
            tc.tile_pool(name="recp", bufs=4) as recp,
            tc.tile_pool(name="work", bufs=1) as work,
            tc.tile_pool(name="psum", bufs=6, space="PSUM") as psum,
        ):
            hs_ap = hs.ap()

            def blk_rows(j):
                k = NBLK - 1 - j
                return hs_ap[k * P : (k + 1) * P, :, :]

            # HWDGE rings crawl (~50-100 GB/s) once the SWDGE flood starts,
            # so the 1 MiB fp16 wrep rides the SWDGE queue itself, in front
            # of the stream (lands ~12 us).  Only the tiny wbc and the
            # late-needed hst31 prefetch use the sync ring; tri/onesq ride
            # the scalar ring ahead of everything they gate.
            wrep_t = consts.tile([P, BC * H], F16)
            nc.gpsimd.dma_start(out=wrep_t, in_=wrep.ap())
            tri_t = consts.tile([P, P], F32)
            nc.scalar.dma_start(out=tri_t, in_=tri.ap())
            onesq_t = consts.tile([P, P], F32)
            nc.scalar.dma_start(out=onesq_t, in_=onesq.ap())
            wrep_v = wrep_t.rearrange("p (b h) -> p b h", h=H)

            # SWDGE fp16 stream; j=0 and j=31 split into two 4-col chunks
            # (short ramp / short tail chains).
            h2 = BC // 2
            dmas = {}
            hst0 = hsp.tile([P, BC, H], F16, name="hst")
            nc.gpsimd.dma_start(out=hst0[:, :h2, :], in_=blk_rows(0)[:, :h2, :])
            nc.gpsimd.dma_start(out=hst0[:, h2:, :], in_=blk_rows(0)[:, h2:, :])
            dmas[0] = hst0
            for j in range(1, NBLK - 1):
                hst = hsp.tile([P, BC, H], F16, name="hst")
                nc.gpsimd.dma_start(out=hst, in_=blk_rows(j))
                dmas[j] = hst
            hst31 = hsp.tile([P, BC, H], F16, name="hst")
            nc.gpsimd.dma_start(out=hst31[:, :h2, :], in_=blk_rows(31)[:, :h2, :])
            nc.gpsimd.dma_start(out=hst31[:, h2:, :], in_=blk_rows(31)[:, h2:, :])
            dmas[31] = hst31

            dummy = work.tile([P, 1], F32)
            esum = [
                work.tile([P, BC], F32, name="esum0"),
                work.tile([P, BC], F32, name="esum1"),
            ]
            nc.vector.memset(esum[0], 0.0)
            sel = work.tile([P, C], F32)
            out_ap = out.ap()

            def scan_cols(j, e_t, lo, hi, last=False):
                """Scan chain for cols [lo,hi) of processed block j."""
                n = hi - lo
                ps = psum.tile([P, n], F32, name="ps")
                nc.tensor.matmul(ps, tri_t, e_t[:, lo:hi], start=True, stop=False)
                nc.tensor.matmul(
                    ps, onesq_t, esum[j % 2][:, lo:hi], start=False, stop=True
                )
                if not last:
                    nc.vector.tensor_add(
                        esum[(j + 1) % 2][:, lo:hi], esum[j % 2][:, lo:hi],
                        e_t[:, lo:hi],
                    )
                rec = recp.tile([P, n], F32, name="rec")
                nc.vector.reciprocal_approx_fast(out=rec, in_=ps)
                c0 = j * BC + lo
                nc.vector.tensor_mul(sel[:, c0 : c0 + n], e_t[:, lo:hi], rec)

            prods = {}

            def emit_mult(j, lo=0, hi=BC):
                prod = prodp.tile([P, BC, H], F16, name="prod")
                nc.vector.tensor_tensor(
                    prod[:, lo:hi, :], dmas[j][:, lo:hi, :], wrep_v[:, lo:hi, :],
                    op=MULT,
                )
                prods[j] = prod

            def process_f16_block(j, act_cols):
                prod = prods.pop(j)
                s_col = scolp.tile([P, BC], F32, name="s_col")
                for b in range(act_cols):
                    nc.scalar.activation(
                        dummy.broadcast_to((P, H)),
                        prod[:, b, :],
                        Copy,
                        accum_out=s_col[:, b : b + 1],
                    )
                if act_cols < BC:
                    nc.vector.reduce_sum(
                        out=s_col[:, act_cols:],
                        in_=prod[:, act_cols:, :],
                        axis=X,
                    )
                e_t = ep.tile([P, BC], F32, name="e_t")
                nc.scalar.activation(e_t, s_col, Exp)
                scan_cols(j, e_t, 0, BC)

            # j=0 ramp block: fp16, processed in two 4-col pieces so ACT/DVE
            # start on the first landed chunk.
            emit_mult(0, 0, h2)
            prod0 = prods[0]
            s_col0 = scolp.tile([P, BC], F32, name="s_col")
            e_t0 = ep.tile([P, BC], F32, name="e_t")
            for pc in range(2):
                lo, hi = pc * h2, (pc + 1) * h2
                if pc:
                    nc.vector.tensor_tensor(
                        prod0[:, lo:hi, :], dmas[0][:, lo:hi, :],
                        wrep_v[:, lo:hi, :], op=MULT,
                    )
                for b in range(lo, lo + 2):
                    nc.scalar.activation(
                        dummy.broadcast_to((P, H)), prod0[:, b, :], Copy,
                        accum_out=s_col0[:, b : b + 1],
                    )
                nc.vector.reduce_sum(
                    out=s_col0[:, lo + 2 : hi], in_=prod0[:, lo + 2 : hi, :], axis=X
                )
                nc.scalar.activation(e_t0[:, lo:hi], s_col0[:, lo:hi], Exp)
                scan_cols(0, e_t0, lo, hi)
            prods.pop(0)

            # middle fp16 blocks, multiplies emitted two blocks ahead so the
            # next block's ACT accums never queue behind this block's scan.
            emit_mult(1)
            emit_mult(2)
            for j in range(1, NBLK - 1):
                if j + 2 <= NBLK - 2:
                    emit_mult(j + 2)
                process_f16_block(j, act_cols=5 if j % 2 else 4)

            # j=31 tail: two 4-col pieces so the post-stream chain is short.
            s31c = scolp.tile([P, BC], F32, name="s_col")
            e31c = ep.tile([P, BC], F32, name="e_t")
            for pc in range(2):
                lo, hi = pc * h2, (pc + 1) * h2
                prod31 = prodp.tile([P, BC, H], F16, name="prod")
                nc.vector.tensor_tensor(
                    prod31[:, lo:hi, :], hst31[:, lo:hi, :], wrep_v[:, lo:hi, :],
                    op=MULT,
                )
                for b in range(lo, lo + 2):
                    nc.scalar.activation(
                        dummy.broadcast_to((P, H)), prod31[:, b, :], Copy,
                        accum_out=s31c[:, b : b + 1],
                    )
                nc.vector.reduce_sum(
                    out=s31c[:, lo + 2 : hi], in_=prod31[:, lo + 2 : hi, :], axis=X
                )
                nc.scalar.activation(e31c[:, lo:hi], s31c[:, lo:hi], Exp)
                scan_cols(31, e31c, lo, hi, last=True)

            for glo, ghi in ((0, 16), (16, 24), (24, 30), (30, 32)):
                lo, hi = glo * BC, ghi * BC
                nc.sync.dma_start(out=out_ap[:, lo:hi], in_=sel[:, lo:hi])

    nc.compile()
    return nc


_PROGRAM = None


def _get_program():
    global _PROGRAM
    if _PROGRAM is None:
        _PROGRAM = build_program()
    return _PROGRAM


def make_in_maps(hidden_states, W):
    hidden_states = np.asarray(hidden_states, dtype=np.float32)
    w16 = np.asarray(W, dtype=np.float32)[:, 0].astype(np.float16)
    wrep = np.ascontiguousarray(np.tile(w16[None, :], (P, BC)))
    tri = np.tril(np.ones((P, P), dtype=np.float32))
    onesq = np.ones((P, P), dtype=np.float32)
    in_maps = []
    for c in range(N_CORES):
        hs_c = np.ascontiguousarray(hidden_states[:, c * BC : (c + 1) * BC, :])
        in_maps.append(
            {"hs": hs_c, "wrep": wrep, "tri": tri, "onesq": onesq}
        )
    return in_maps


def assemble_output(results):
    cols = []
    for c in range(N_CORES):
        oc = results[c]["out"]
        full = oc.reshape(P, NBLK, BC)[:, ::-1, :].transpose(1, 0, 2).reshape(S, BC)
        cols.append(full)
    return np.concatenate(cols, axis=1)[..., None].astype(np.float32)


def kernel(hidden_states, W, b):
    nc = _get_program()
    in_maps = make_in_maps(hidden_states, W)
    res = bass_utils.run_bass_kernel_spmd(nc, in_maps, core_ids=list(range(N_CORES)))
    return assemble_output(res.results)
